# revision 2
# baseline (speedup 1.0000x reference)
"""Trainium2 8-core attention kernel (B=2, N=2048, D=1024, H=16).

Sharding: core c = 4*b + g handles batch b, query rows [g*512, (g+1)*512),
all 16 heads. Each core receives the full x^T of its batch with sequence
blocks rotated so its own block sits at column 0 (keys are permutation-
invariant under softmax). Duo 0 (heads 0-1) computes K/V locally over the
whole sequence; duos 1-7 K/V shards AllGather in chunks [1,2,2,2] that
overlap attention.

v4: attention starts ~12us (duo0-local prologue); PV is col-tiled (two
concurrent M=64 matmuls, packed o_acc in one PSUM bank); softmax
denominators come from a DVE bf16 running sum of the exp tiles, reduced
across partitions by 8 tiny K-column matmuls; all remaining QKV/staging/
proj PE work is interleaved into the ACT-paced attention loop as fillers.
"""

import sys

if "/opt/trn_rl_repo" not in sys.path:
    sys.path.insert(0, "/opt/trn_rl_repo")

import numpy as np
import ml_dtypes

import concourse.bass as bass
import concourse.mybir as mybir
from concourse import bacc, tile
from concourse import bass_utils

FP32 = mybir.dt.float32
BF16 = mybir.dt.bfloat16

B, N, D = 2, 2048, 1024
H, HD = 16, 64
SCALE = HD ** -0.5
NC = 8
GROUPS = [[0, 1, 2, 3], [4, 5, 6, 7]]
NQ = N // 4          # query rows per core (512)
KT = N // 128        # key k-tiles (16)
CT = D // 128        # 128-channel tiles per D (8)
CHUNKS = [1, 2, 2, 2]                # AllGather chunks (duos), for duos 1..7
CH_OFF = [sum(CHUNKS[:i]) for i in range(len(CHUNKS) + 1)]
KSZ = 128 * NQ                       # per-duo K^T elems (65536)
DUO_ELEMS = 2 * KSZ                  # per-duo: K^T [128,512] + V [512,128]

_compiled = None


def build():
    from contextlib import ExitStack

    nc = bacc.Bacc("TRN2", target_bir_lowering=False, debug=False, num_devices=NC)

    xT = nc.dram_tensor("xT", [D, N], BF16, kind="ExternalInput")
    w_qkv = nc.dram_tensor("w_qkv", [D, 3 * D], BF16, kind="ExternalInput")
    w_proj = nc.dram_tensor("w_proj", [D, D], BF16, kind="ExternalInput")
    b_qk = nc.dram_tensor("b_qk", [128, 16], FP32, kind="ExternalInput")
    b_v = nc.dram_tensor("b_v", [128, D], FP32, kind="ExternalInput")
    b_prj = nc.dram_tensor("b_prj", [128, D], FP32, kind="ExternalInput")
    ident = nc.dram_tensor("ident", [128, 128], BF16, kind="ExternalInput")
    out = nc.dram_tensor("out", [NQ, D], FP32, kind="ExternalOutput")

    with tile.TileContext(nc) as tc, ExitStack() as ctx:
        if True:
            bias_pool = ctx.enter_context(tc.tile_pool(name="bias", bufs=4))
            cst_pool = ctx.enter_context(tc.tile_pool(name="cst", bufs=5))
            xto_pool = ctx.enter_context(tc.tile_pool(name="xto", bufs=8))
            xtr_pool = ctx.enter_context(tc.tile_pool(name="xtr", bufs=24))
            wkl_pool = ctx.enter_context(tc.tile_pool(name="wkl", bufs=8))
            wvl_pool = ctx.enter_context(tc.tile_pool(name="wvl", bufs=8))
            wql_pool = ctx.enter_context(tc.tile_pool(name="wql", bufs=8))
            wka_pool = ctx.enter_context(tc.tile_pool(name="wka", bufs=8))
            wva_pool = ctx.enter_context(tc.tile_pool(name="wva", bufs=8))
            wqr_pool = ctx.enter_context(tc.tile_pool(name="wqr", bufs=8))
            wp_pool = ctx.enter_context(tc.tile_pool(name="wp", bufs=8))
            qt_pool = ctx.enter_context(tc.tile_pool(name="qt", bufs=8))
            ktp_pool = ctx.enter_context(tc.tile_pool(name="ktp", bufs=4))
            vsb_pool = ctx.enter_context(tc.tile_pool(name="vsb", bufs=4))
            es_pool = ctx.enter_context(tc.tile_pool(name="es", bufs=4))
            den_pool = ctx.enter_context(tc.tile_pool(name="den", bufs=2))
            stg_pool = ctx.enter_context(tc.tile_pool(name="stg", bufs=3))
            nrm_pool = ctx.enter_context(tc.tile_pool(name="nrm", bufs=3))
            rr_pool = ctx.enter_context(tc.tile_pool(name="rr", bufs=6))
            ot_pool = ctx.enter_context(tc.tile_pool(name="ot", bufs=8))
            y_pool = ctx.enter_context(tc.tile_pool(name="yy", bufs=4))
            ps1 = ctx.enter_context(tc.tile_pool(name="ps1", bufs=2, space="PSUM"))
            psS = ctx.enter_context(tc.tile_pool(name="psS", bufs=2, space="PSUM"))
            psO = ctx.enter_context(tc.tile_pool(name="psO", bufs=2, space="PSUM"))
            dram = ctx.enter_context(tc.tile_pool(name="dram", bufs=1, space="DRAM"))

            # ---- warmups: ACT exp table, HAM window, collective bootstrap
            aw_in = cst_pool.tile([1, 2], FP32, tag="aw", name="awi", bufs=1)
            nc.vector.memset(aw_in[:], 0.0)
            aw_out = cst_pool.tile([1, 2], BF16, tag="aw2", name="awo", bufs=1)
            nc.scalar.activation(
                aw_out[:], aw_in[:], mybir.ActivationFunctionType.Exp
            )

            scr = cst_pool.tile([128, 512], BF16, tag="scr", name="scr", bufs=1)
            nc.vector.memset(scr[:], 0.0)
            ps_w = ps1.tile([128, 512], FP32, tag="acc", name="warm")
            for i in range(8):
                nc.tensor.matmul(
                    ps_w[:], scr[:, 0:128], scr[:],
                    start=(i == 0), stop=(i == 7),
                )

            warm_in = dram.tile([64], BF16, tag="wmi")
            warm_out = dram.tile([256], BF16, tag="wmo")
            wsb = cst_pool.tile([1, 64], BF16, tag="wsb", name="wsb", bufs=1)
            nc.vector.memset(wsb[:], 0.0)
            nc.gpsimd.dma_start(warm_in.rearrange("(a x) -> a x", a=1), wsb[:])
            nc.gpsimd.collective_compute(
                "AllGather", mybir.AluOpType.bypass, replica_groups=GROUPS,
                ins=[warm_in.opt()], outs=[warm_out.opt()],
            )

            ones64 = cst_pool.tile([1, HD], BF16, tag="onef", name="ones64",
                                   bufs=1)
            nc.vector.memset(ones64[:], 1.0)
            ones128 = cst_pool.tile([128, 1], BF16, tag="onec", name="ones128",
                                    bufs=1)
            nc.vector.memset(ones128[:], 1.0)

            ident_sb = bias_pool.tile([128, 128], BF16, tag="idn", name="idn",
                                      bufs=1)
            nc.sync.dma_start(ident_sb[:], ident.ap()[:])

            # ---- input DMAs, latency-ordered (sync queue = HWDGE) ----
            bqk_sb = bias_pool.tile([128, 16], FP32, tag="bias")
            nc.sync.dma_start(bqk_sb[:], b_qk.ap()[:])
            bv_sb = bias_pool.tile([128, D], FP32, tag="bias")
            nc.sync.dma_start(bv_sb[:], b_v.ap()[:])

            xt_own = []
            for k in range(CT):
                t = xto_pool.tile([128, NQ], BF16, tag="xto", name=f"xo{k}")
                nc.sync.dma_start(t[:], xT.ap()[k * 128:(k + 1) * 128, 0:NQ])
                xt_own.append(t)

            # duo0 K/V weight columns + Q0 weight columns
            wkl, wvl, wql = [], [], []
            for k in range(CT):
                t = wkl_pool.tile([128, 128], BF16, tag="wkl", name=f"wkl{k}")
                nc.sync.dma_start(t[:], w_qkv.ap()[k * 128:(k + 1) * 128,
                                                   D:D + 128])
                wkl.append(t)
                t = wvl_pool.tile([128, 128], BF16, tag="wvl", name=f"wvl{k}")
                nc.sync.dma_start(t[:], w_qkv.ap()[k * 128:(k + 1) * 128,
                                                   2 * D:2 * D + 128])
                wvl.append(t)
                t = wql_pool.tile([128, 128], BF16, tag="wql", name=f"wql{k}")
                nc.sync.dma_start(t[:], w_qkv.ap()[k * 128:(k + 1) * 128,
                                                   0:128])
                wql.append(t)

            # rest of x^T, per seq block r=1..3 so dependencies are fine
            xtr = {r: [] for r in (1, 2, 3)}
            for r in (1, 2, 3):
                for k in range(CT):
                    t = xtr_pool.tile([128, NQ], BF16, tag="xtr",
                                      name=f"xr{r}_{k}")
                    nc.sync.dma_start(
                        t[:], xT.ap()[k * 128:(k + 1) * 128,
                                      r * NQ:(r + 1) * NQ])
                    xtr[r].append(t)

            def xt_cols(k, c0, c1):
                # xT columns [c0, c1) (must not straddle a 512 block)
                r = c0 // NQ
                if r == 0:
                    return xt_own[k][:, c0:c1]
                return xtr[r][k][:, c0 - r * NQ:c1 - r * NQ]

            # AG-duo K/V weight columns, chunk-major
            wka, wva = [], []
            for c, nduo in enumerate(CHUNKS):
                d0 = 1 + CH_OFF[c]
                ks, vs = [], []
                for k in range(CT):
                    t = wka_pool.tile([128, 128 * nduo], BF16, tag=f"wka{c}",
                                      name=f"wka{c}_{k}")
                    nc.sync.dma_start(
                        t[:], w_qkv.ap()[k * 128:(k + 1) * 128,
                                         D + d0 * 128:D + (d0 + nduo) * 128])
                    ks.append(t)
                    t = wva_pool.tile([128, 128 * nduo], BF16, tag=f"wva{c}",
                                      name=f"wva{c}_{k}")
                    nc.sync.dma_start(
                        t[:], w_qkv.ap()[k * 128:(k + 1) * 128,
                                         2 * D + d0 * 128:
                                         2 * D + (d0 + nduo) * 128])
                    vs.append(t)
                wka.append(ks)
                wva.append(vs)

            # Q weight columns for duos 1-7
            wqr = []
            for k in range(CT):
                t = wqr_pool.tile([128, D - 128], BF16, tag="wqr",
                                  name=f"wqr{k}")
                nc.sync.dma_start(t[:], w_qkv.ap()[k * 128:(k + 1) * 128,
                                                   128:D])
                wqr.append(t)

            # ---- DRAM bounce + AG buffers, duo-major ----
            kv_in = dram.tile([7 * DUO_ELEMS], BF16, tag="kvin")
            kv_ag = dram.tile([4 * 7 * DUO_ELEMS], BF16, tag="kvag")

            duo_tiles = {}
            qt = [None] * (H // 2)
            ot = []

            # ---- emit helpers ----
            def emit_k_local(r):
                ktp0 = duo_tiles[0][0]
                ps = ps1.tile([128, NQ], FP32, tag="acc", name=f"psKL{r}")
                for k in range(CT):
                    nc.tensor.matmul(
                        ps[:], wkl[k][:], xt_cols(k, r * NQ, (r + 1) * NQ),
                        start=(k == 0), stop=(k == CT - 1),
                    )
                nc.vector.tensor_scalar_add(
                    ktp0[:, r * NQ:(r + 1) * NQ], ps[:], bqk_sb[:, 8:9]
                )

            def emit_v_local(m):
                va0 = duo_tiles[0][1]
                ps = ps1.tile([128, 128], FP32, tag="acc", name=f"psVL{m}")
                for k in range(CT):
                    nc.tensor.matmul(
                        ps[:], xt_cols(k, m * 128, (m + 1) * 128), wvl[k][:],
                        start=(k == 0), stop=(k == CT - 1),
                    )
                nc.vector.scalar_tensor_tensor(
                    va0[:, m * 128:(m + 1) * 128], ps[:], 0.0,
                    bv_sb[:, 0:128],
                    op0=mybir.AluOpType.bypass, op1=mybir.AluOpType.add,
                )

            def emit_qt(d):
                ps = ps1.tile([128, NQ], FP32, tag="acc", name=f"psQ{d}")
                for k in range(CT):
                    w = (wql[k][:] if d == 0
                         else wqr[k][:, (d - 1) * 128:d * 128])
                    nc.tensor.matmul(
                        ps[:], w, xt_own[k][:],
                        start=(k == 0), stop=(k == CT - 1),
                    )
                sb = qt_pool.tile([128, NQ], BF16, tag="qt", name=f"qt{d}")
                nc.vector.tensor_scalar_add(sb[:], ps[:], bqk_sb[:, d:d + 1])
                qt[d] = sb

            # AG staging: K^T and V of own block for duo d (in chunk c, pos i)
            def emit_stage_k(c, i):
                d = 1 + CH_OFF[c] + i
                ps = ps1.tile([128, NQ], FP32, tag="acc", name=f"psSK{d}")
                for k in range(CT):
                    nc.tensor.matmul(
                        ps[:], wka[c][k][:, i * 128:(i + 1) * 128],
                        xt_own[k][:],
                        start=(k == 0), stop=(k == CT - 1),
                    )
                sb = stg_pool.tile([128, NQ], BF16, tag="stg", name=f"ksb{d}")
                nc.vector.tensor_scalar_add(sb[:], ps[:], bqk_sb[:, 8 + d:9 + d])
                kin = kv_in[(d - 1) * DUO_ELEMS:
                            (d - 1) * DUO_ELEMS + KSZ].rearrange(
                    "(p q) -> p q", q=NQ)
                nc.gpsimd.dma_start(kin, sb[:])

            def emit_stage_v(c, i):
                d = 1 + CH_OFF[c] + i
                sbv = stg_pool.tile([128, NQ], BF16, tag="stg", name=f"vsb{d}")
                for m in range(4):
                    ps = ps1.tile([128, 128], FP32, tag="acc",
                                  name=f"psSV{d}{m}")
                    for k in range(CT):
                        nc.tensor.matmul(
                            ps[:], xt_own[k][:, m * 128:(m + 1) * 128],
                            wva[c][k][:, i * 128:(i + 1) * 128],
                            start=(k == 0), stop=(k == CT - 1),
                        )
                    nc.vector.scalar_tensor_tensor(
                        sbv[:, m * 128:(m + 1) * 128], ps[:], 0.0,
                        bv_sb[:, d * 128:(d + 1) * 128],
                        op0=mybir.AluOpType.bypass, op1=mybir.AluOpType.add,
                    )
                vin = kv_in[(d - 1) * DUO_ELEMS + KSZ:
                            d * DUO_ELEMS].rearrange("(p x) -> p x", x=NQ)
                nc.gpsimd.dma_start(vin, sbv[:])

            def emit_trigger(c):
                nduo = CHUNKS[c]
                base = CH_OFF[c] * DUO_ELEMS
                nc.gpsimd.collective_compute(
                    "AllGather", mybir.AluOpType.bypass, replica_groups=GROUPS,
                    ins=[kv_in[base:base + nduo * DUO_ELEMS].opt()],
                    outs=[kv_ag[4 * base:4 * (base + nduo * DUO_ELEMS)].opt()],
                )

            def emit_loadback(c):
                nduo = CHUNKS[c]
                base = CH_OFF[c] * DUO_ELEMS
                blk = kv_ag[4 * base:4 * (base + nduo * DUO_ELEMS)]
                for i in range(nduo):
                    d = 1 + CH_OFF[c] + i
                    ktp = ktp_pool.tile([128, N], BF16, tag="ktp",
                                        name=f"ktp{d}")
                    src = bass.AP(
                        blk.tensor, blk.offset + i * DUO_ELEMS,
                        [[NQ, 128], [nduo * DUO_ELEMS, 4], [1, NQ]],
                    )
                    nc.sync.dma_start(
                        ktp[:].rearrange("p (r q) -> p r q", r=4), src)
                    va = vsb_pool.tile([128, KT * 2 * HD], BF16, tag="vsb",
                                       name=f"va{d}")
                    va5 = va[:].rearrange(
                        "cc (r sh j e) -> cc r sh j e", r=4, sh=4, j=2, e=HD)
                    for r in range(4):
                        src = bass.AP(
                            blk.tensor,
                            blk.offset + (r * nduo + i) * DUO_ELEMS + KSZ,
                            [[NQ, 128], [128, 4], [64, 2], [1, HD]],
                        )
                        nc.sync.dma_start(va5[:, r, :, :, :], src)
                    duo_tiles[d] = (ktp, va)

            # ---- prologue: duo0-local K/V/Q over own block + r1 ----
            ktp0 = ktp_pool.tile([128, N], BF16, tag="ktp", name="ktp0")
            va0 = vsb_pool.tile([128, KT * 2 * HD], BF16, tag="vsb",
                                name="va0")
            duo_tiles[0] = (ktp0, va0)

            emit_k_local(0)
            for m in range(4):
                emit_v_local(m)
            emit_qt(0)
            emit_k_local(1)

            # ---- filler schedule: {(duo, kt): [thunks]} ----
            def thunk(f, *a):
                return lambda: f(*a)

            def stage_kv_trig_lb(c, i, last):
                def run():
                    emit_stage_v(c, i)
                    if last:
                        emit_trigger(c)
                        emit_loadback(c)
                return run

            fillers = {}

            def add(d, kt, th):
                fillers.setdefault((d, kt), []).append(th)

            add(0, 0, thunk(emit_stage_k, 0, 0))
            add(0, 1, stage_kv_trig_lb(0, 0, True))
            add(0, 2, thunk(emit_v_local, 4))
            add(0, 3, thunk(emit_v_local, 5))
            add(0, 4, thunk(emit_v_local, 6))
            add(0, 4, thunk(emit_k_local, 2))
            add(0, 5, thunk(emit_v_local, 7))
            add(0, 6, thunk(emit_v_local, 8))
            add(0, 7, thunk(emit_v_local, 9))
            add(0, 7, thunk(emit_k_local, 3))
            add(0, 8, thunk(emit_v_local, 10))
            add(0, 9, thunk(emit_v_local, 11))
            add(0, 10, thunk(emit_v_local, 12))
            add(0, 11, thunk(emit_v_local, 13))
            add(0, 11, thunk(emit_qt, 1))
            add(0, 12, thunk(emit_v_local, 14))
            add(0, 13, thunk(emit_v_local, 15))
            add(0, 14, thunk(emit_stage_k, 1, 0))
            add(1, 0, stage_kv_trig_lb(1, 0, False))
            add(1, 1, thunk(emit_stage_k, 1, 1))
            add(1, 2, stage_kv_trig_lb(1, 1, True))
            add(1, 3, thunk(emit_stage_k, 2, 0))
            add(1, 4, stage_kv_trig_lb(2, 0, False))
            add(1, 5, thunk(emit_stage_k, 2, 1))
            add(1, 6, stage_kv_trig_lb(2, 1, True))
            add(1, 8, thunk(emit_qt, 2))
            add(2, 0, thunk(emit_stage_k, 3, 0))
            add(2, 1, stage_kv_trig_lb(3, 0, False))
            add(2, 2, thunk(emit_stage_k, 3, 1))
            add(2, 3, stage_kv_trig_lb(3, 1, True))
            add(2, 8, thunk(emit_qt, 3))
            for d in range(3, 7):
                add(d, 8, thunk(emit_qt, d + 1))

            # ---- softmax denominator reduce / normalize phases ----
            def reduce_phase(den, d):
                den_ps = ps1.tile([128, 8], FP32, tag="acc", name=f"dn{d}")
                for qc in range(8):
                    nc.tensor.matmul(
                        den_ps[:, qc:qc + 1],
                        den[:, qc * 128:(qc + 1) * 128], ones128[:],
                        start=True, stop=True,
                    )
                rr_sb = rr_pool.tile([128, 8], BF16, tag="rrs", name=f"rs{d}")
                with nc.allow_low_precision(reason="softmax denom recip bf16"):
                    nc.vector.reciprocal(rr_sb[:], den_ps[:])
                rrT_ps = ps1.tile([8, 128], BF16, tag="acc", name=f"rt{d}")
                nc.tensor.transpose(rrT_ps[:], rr_sb[:], ident_sb[:])
                rrT_sb = rr_pool.tile([8, 128], BF16, tag="rrt", name=f"rt{d}")
                nc.vector.tensor_copy(rrT_sb[:], rrT_ps[:])
                rr_row = rr_pool.tile([1, 2 * NQ], BF16, tag="rrr",
                                      name=f"rr{d}")
                nc.gpsimd.dma_start(
                    rr_row[:].rearrange("a (r q) -> a r q", r=8),
                    rrT_sb[:].rearrange("r (b q) -> r b q", b=1),
                )
                return rr_row

            def apply_phase(o_sb, rr_row, d):
                rbp = ps1.tile([128, NQ], FP32, tag="acc", name=f"rbp{d}")
                for j in range(2):
                    nc.tensor.matmul(
                        rbp[j * HD:(j + 1) * HD, :], ones64[:],
                        rr_row[:, j * NQ:(j + 1) * NQ],
                        start=True, stop=True,
                    )
                otd = ot_pool.tile([128, NQ], BF16, tag="ot", name=f"ot{d}")
                nc.vector.scalar_tensor_tensor(
                    otd[:], o_sb[:], 0.0, rbp[:],
                    op0=mybir.AluOpType.bypass, op1=mybir.AluOpType.mult,
                )
                ot.append(otd)

            # ---- attention: 8 duos, software-pipelined ----
            prev_last_sc = None
            pend = []          # (o_sb, den, duo idx) awaiting normalization
            red = {}           # duo idx -> rr_row

            for d in range(H // 2):
                ktp, va = duo_tiles[d]
                o_acc = psO.tile([128, NQ], FP32, tag="oac", name=f"oacc{d}")
                den = den_pool.tile([128, 2 * NQ], BF16, tag="den",
                                    name=f"den{d}")
                es_tiles = [None] * KT

                def emit_pv(kt_i):
                    es_kt = es_tiles[kt_i]
                    for j in range(2):
                        nc.tensor.matmul(
                            o_acc[j * HD:(j + 1) * HD, :],
                            va[:, kt_i * 128 + j * HD:
                               kt_i * 128 + (j + 1) * HD],
                            es_kt[:, j * NQ:(j + 1) * NQ],
                            start=(kt_i == 0), stop=(kt_i == KT - 1),
                        )

                for kt in range(KT):
                    s = psS.tile([128, 2 * NQ], FP32, tag="squad",
                                 name=f"s{d}_{kt}")
                    for i in range(2):
                        mm = nc.tensor.matmul(
                            s[:, i * NQ:(i + 1) * NQ],
                            ktp[i * HD:(i + 1) * HD, kt * 128:(kt + 1) * 128],
                            qt[d][i * HD:(i + 1) * HD, :],
                            start=True, stop=True,
                        )
                        if kt == 0 and i == 0 and prev_last_sc is not None:
                            bass._add_dep_helper(
                                mm.ins, prev_last_sc.ins, sync=False,
                                reason="keep PE duo-sequential",
                            )
                        last_sc = mm
                    es = es_pool.tile([128, 2 * NQ], BF16, tag="es",
                                      name=f"es{d}_{kt}")
                    nc.scalar.activation(
                        es[:], s[:], mybir.ActivationFunctionType.Exp,
                        scale=SCALE,
                    )
                    es_tiles[kt] = es
                    if kt == 0:
                        nc.vector.tensor_copy(den[:], es[:])
                    else:
                        nc.vector.tensor_tensor(
                            den[:], den[:], es[:], op=mybir.AluOpType.add)
                    if kt >= 1:
                        emit_pv(kt - 1)
                    if kt == 2 and pend:
                        o_p, den_p, d_p = pend.pop(0)
                        red[d_p] = (o_p, reduce_phase(den_p, d_p), d_p)
                    elif kt == 12 and red:
                        o_p, rr_p, d_p = red.pop(d - 1)
                        apply_phase(o_p, rr_p, d_p)
                    for th in fillers.get((d, kt), []):
                        th()
                emit_pv(KT - 1)
                prev_last_sc = last_sc
                o_sb = nrm_pool.tile([128, NQ], BF16, tag="oc", name=f"oc{d}")
                with nc.allow_low_precision(reason="unnorm O to bf16"):
                    nc.vector.tensor_copy(o_sb[:], o_acc[:])
                pend.append((o_sb, den, d))

            # ---- W_proj + projection bias (sync queue, after loadbacks) ----
            wp = []
            for k in range(CT):
                t = wp_pool.tile([128, D], BF16, tag="wp", name=f"wp{k}")
                nc.sync.dma_start(t[:], w_proj.ap()[k * 128:(k + 1) * 128, :])
                wp.append(t)
            bp_sb = bias_pool.tile([128, D], FP32, tag="bias")
            nc.sync.dma_start(bp_sb[:], b_prj.ap()[:])

            # ---- output projection: wave A holds 4 accumulators (2 psS +
            # 2 psO, both idle now) over ot[0..6] while the last duo's
            # normalization chain is in flight; ps1 stays free for it.
            MN = [(m, n) for m in range(NQ // 128) for n in range(D // 512)]
            proj_ps = []
            for idx, (m, n) in enumerate(MN[:4]):
                pool = psS if idx < 2 else psO
                ps = pool.tile([128, 512], FP32,
                               tag="squad" if idx < 2 else "oac",
                               name=f"psP{m}{n}")
                for k in range(CT - 1):
                    nc.tensor.matmul(
                        ps[:], ot[k][:, m * 128:(m + 1) * 128],
                        wp[k][:, n * 512:(n + 1) * 512],
                        start=(k == 0), stop=False,
                    )
                proj_ps.append((ps, m, n))

            o_l, den_l, d_l = pend.pop(0)
            apply_phase(o_l, reduce_phase(den_l, d_l), d_l)

            def proj_finish(ps, m, n):
                nc.tensor.matmul(
                    ps[:], ot[CT - 1][:, m * 128:(m + 1) * 128],
                    wp[CT - 1][:, n * 512:(n + 1) * 512],
                    start=False, stop=True,
                )
                y = y_pool.tile([128, 512], FP32, tag="yy", name=f"y{m}{n}")
                nc.vector.scalar_tensor_tensor(
                    y[:], ps[:], 0.0, bp_sb[:, n * 512:(n + 1) * 512],
                    op0=mybir.AluOpType.bypass, op1=mybir.AluOpType.add,
                )
                nc.sync.dma_start(
                    out.ap()[m * 128:(m + 1) * 128, n * 512:(n + 1) * 512],
                    y[:],
                )

            for ps, m, n in proj_ps:
                proj_finish(ps, m, n)
            for m, n in MN[4:]:
                ps = ps1.tile([128, 512], FP32, tag="acc", name=f"psP{m}{n}")
                for k in range(CT - 1):
                    nc.tensor.matmul(
                        ps[:], ot[k][:, m * 128:(m + 1) * 128],
                        wp[k][:, n * 512:(n + 1) * 512],
                        start=(k == 0), stop=False,
                    )
                proj_finish(ps, m, n)

    nc.compile()
    return nc


def make_in_maps(x, W_qkv, b_qkv, W_proj, b_proj):
    x = np.asarray(x, dtype=np.float32)
    W_qkv = np.asarray(W_qkv, dtype=np.float32)
    b_qkv = np.asarray(b_qkv, dtype=np.float32)
    W_proj = np.asarray(W_proj, dtype=np.float32)
    b_proj = np.asarray(b_proj, dtype=np.float32)

    wq_bf = W_qkv.astype(ml_dtypes.bfloat16)
    wp_bf = W_proj.astype(ml_dtypes.bfloat16)
    bqk = np.ascontiguousarray(b_qkv[:2 * D].reshape(16, 128).T)
    bv = np.tile(b_qkv[2 * D:], (128, 1)).astype(np.float32)
    bp = np.tile(b_proj, (128, 1)).astype(np.float32)

    in_maps = []
    for c in range(NC):
        b, g = divmod(c, 4)
        xt_rot = np.concatenate(
            [x[b, ((g + i) % 4) * NQ:(((g + i) % 4) + 1) * NQ, :].T
             for i in range(4)], axis=1
        )
        in_maps.append({
            "xT": np.ascontiguousarray(xt_rot).astype(ml_dtypes.bfloat16),
            "w_qkv": wq_bf,
            "w_proj": wp_bf,
            "b_qk": bqk,
            "b_v": bv,
            "b_prj": bp,
            "ident": np.eye(128, dtype=ml_dtypes.bfloat16),
        })
    return in_maps


def run(inputs, trace=False):
    global _compiled
    if _compiled is None:
        _compiled = build()
    in_maps = make_in_maps(**inputs)
    res = bass_utils.run_bass_kernel_spmd(
        _compiled, in_maps, core_ids=list(range(NC)), trace=trace
    )
    full = np.empty((B, N, D), dtype=np.float32)
    for c in range(NC):
        b, g = divmod(c, 4)
        full[b, g * NQ:(g + 1) * NQ, :] = res.results[c]["out"]
    return full, res


def kernel(x, W_qkv, b_qkv, W_proj, b_proj):
    full, _ = run(dict(x=x, W_qkv=W_qkv, b_qkv=b_qkv, W_proj=W_proj, b_proj=b_proj))
    return full
